# revision 1
# baseline (speedup 1.0000x reference)
"""DifferentiableLengthRegulator Trainium2 kernel.

out[b,c,l] = y_mask * (sum_t x[b,c,t]*W[b,t,l]) / (sum_t W[b,t,l] + eps)
W = exp(-0.5*(l - center[b,t])^2 / (w[b,t]^2*sigma_scale^2 + eps))

Sharding: data-parallel over batch B=16 -> 8 cores x 2 batches.
Per core, per batch (banded over the frame axis, since the Gaussian
weights vanish outside ~13 sigma of each token chunk's centers):
  DVE : mu = pos - c                        (tensor_scalar, 2x fp32)
  ACT : W  = DerivErf(s*mu) -> bf16         (= 2/sqrt(pi) * exp(-(s*mu)^2);
        the 2/sqrt(pi) factor cancels in the normalization)
  PE  : psum[l,0:257] = sum_tc W_tc[:,l-slice]^T @ [xT | ones]  (bf16)
  DVE/ACT/POOL: rd = y_mask/(psum[:,256]+eps);
        out_sb[l,c] = psum[l,0:256]*rd (PSUM->SBUF move, engine-balanced)
Output written (B, L, C)-contiguous; host returns the transpose view.
"""

import numpy as np
import ml_dtypes

B, C, T, L = 16, 256, 512, 4096
N_CORES = 8
BPC = B // N_CORES  # batches per core
CH = 128            # partition chunk
TCN = T // CH       # 4 token chunks
LCN = L // CH       # 32 frame chunks
GRP = 2             # frame chunks per psum group
NGRP = LCN // GRP   # 16 groups
EPS = 1e-8
MARGIN_SIGMA = 13.19
BAND_ALIGN = 128

_bf16 = ml_dtypes.bfloat16
_cache = {}


def _center_scale(w, sigma_scale):
    """Mirror the reference's cumsum/center math (same jax backend bits)."""
    try:
        import jax.numpy as jnp

        wj = jnp.asarray(w)
        center = np.asarray(jnp.cumsum(wj, axis=1) - 0.5 * wj, dtype=np.float32)
    except Exception:
        center = (np.cumsum(w, axis=1, dtype=np.float32) - 0.5 * w).astype(np.float32)
    sigma = (w * np.float32(sigma_scale)).astype(np.float32)
    # W = DerivErf(s*mu)*sqrt(pi)/2 = exp(-(s*mu)^2), s = sqrt(0.5/(sig^2+eps))
    s = np.sqrt(np.float32(0.5) / (np.square(sigma) + np.float32(EPS))).astype(np.float32)
    return center, s


def _bands(center, w_all):
    """Per (slot, tc) aligned frame band, unioned across cores (SPMD)."""
    margin = float(MARGIN_SIGMA * w_all.max() + 1.0)
    bands = []
    for slot in range(BPC):
        rows = center[slot::BPC]  # the 8 batches that land in this slot
        sb = []
        for tc in range(TCN):
            seg = rows[:, tc * CH:(tc + 1) * CH]
            bs = max(0, int(np.floor((seg.min() - margin) / BAND_ALIGN)) * BAND_ALIGN)
            be = min(L, int(np.ceil((seg.max() + margin) / BAND_ALIGN)) * BAND_ALIGN)
            if tc == 0:
                bs = 0
            if tc == TCN - 1:
                be = L
            bs = min(bs, be - CH)
            sb.append((bs, be))
        bands.append(sb)
    return bands


def _split_excess_waits(nc, max_waits=1):
    """walrus here caps sync-waits at 1 per compute instruction; move the
    excess onto injected same-engine NoOps just before the instruction
    (waiting earlier on the same engine is always safe)."""
    from concourse import mybir

    for f in nc.m.functions:
        for blk in f.blocks:
            new = []
            for inst in blk.instructions:
                si = inst.sync_info
                if si is not None and len(si.on_wait) > max_waits:
                    waits = list(si.on_wait)
                    keep, extra = waits[-max_waits:], waits[:-max_waits]
                    for i in range(0, len(extra), max_waits):
                        nop = mybir.InstNoOp(name=f"{inst.name}-xw{i}", ins=[], outs=[])
                        nop.engine = inst.engine
                        nop.sync_info = mybir.SyncInfo(
                            on_wait=extra[i:i + max_waits], on_update=[])
                        new.append(nop)
                    inst.sync_info = mybir.SyncInfo(
                        on_wait=keep, on_update=list(si.on_update))
                new.append(inst)
            blk.instructions = new


def _slim_tile_exit(tile):
    """Drop the second all-engine barrier in Tile's exit sequence: the
    sem-clears it orders are already completed by each engine finishing its
    own instruction stream before the NEFF ends (~4us saved)."""
    if getattr(tile.TileContext, "_slim_exit", False):
        return
    ScopedClock = tile.ScopedClock

    def _drain_and_barrier(self, tick_clock, wait_clock):
        drain_inst = self.nc.sync.drain()
        wait_clock.add_sem_waits(
            drain_inst.ins, ScopedClock({None: tick_clock.global_clock}))
        self.nc.all_engine_barrier()
        popped = self.nc._tile_sem_poison_stack.pop()
        assert popped is self._sem_poison
        self.nc.clear_and_free_semaphores(list(self.sems.allocated().values()))

    tile.TileContext._drain_and_barrier = _drain_and_barrier
    tile.TileContext._slim_exit = True


def _build(band_key):
    import concourse.bass as bass
    import concourse.tile as tile
    from concourse import mybir

    _slim_tile_exit(tile)
    band_key, trivial_masks = band_key
    bands = [[(band_key[s][t][0], band_key[s][t][1]) for t in range(TCN)]
             for s in range(BPC)]
    wmax = max(be - bs for sb in bands for (bs, be) in sb)

    nc = bass.Bass("TRN2", target_bir_lowering=False, debug=False)
    # xta host layout: [b, p, tc, c] so the DMA is descriptor-light
    xta_d = nc.declare_dram_parameter("xta", [BPC, CH, TCN, C + 1], mybir.dt.bfloat16, isOutput=False)
    coefs_d = nc.declare_dram_parameter("coefs", [CH, 3 * BPC * TCN], mybir.dt.float32, isOutput=False)
    ym_d = nc.declare_dram_parameter("ym", [CH, BPC * LCN], mybir.dt.float32, isOutput=False)
    out_d = nc.declare_dram_parameter("out", [BPC, L, C], mybir.dt.float32, isOutput=True)

    f32 = mybir.dt.float32
    bf16 = mybir.dt.bfloat16
    FT = mybir.ActivationFunctionType
    OP = mybir.AluOpType

    def bcast(ap_col, n):
        return bass.AP(tensor=ap_col.tensor, offset=ap_col.offset,
                       ap=list(ap_col.ap) + [[0, n]])

    with tile.TileContext(nc) as tc_:
        import contextlib

        with contextlib.ExitStack() as ctx:
            consts = ctx.enter_context(tc_.tile_pool(name="consts", bufs=1))
            xta_p = ctx.enter_context(tc_.tile_pool(name="xta", bufs=2))
            mu_p = ctx.enter_context(tc_.tile_pool(name="mu", bufs=3))
            w_pools = [ctx.enter_context(tc_.tile_pool(name=f"w{t}", bufs=2)) for t in range(TCN)]
            psum_p = ctx.enter_context(tc_.tile_pool(name="ps", bufs=4, space="PSUM"))
            small_p = ctx.enter_context(tc_.tile_pool(name="small", bufs=6))
            out_p = ctx.enter_context(tc_.tile_pool(name="osb", bufs=4))

            # --- constants. pos comes entirely from iota on GpSimd (no DMA
            # wait at all); coefs on the sync HWDGE ring (keep ScalarE free
            # for the ACT table load -- dma_start blocks its issuing engine).
            coefs_sb = consts.tile([CH, 3 * BPC * TCN], f32)
            nc.sync.dma_start(out=coefs_sb, in_=coefs_d[:, :])
            pos_f = consts.tile([CH, L], f32)
            IW = max(bands[s][0][1] for s in range(BPC))
            IW2 = max(bands[s][1][1] for s in range(BPC))
            IH = (IW // 2 + CH - 1) // CH * CH
            for lo, hi in ((0, IH), (IH, IW), (IW, IW2)):
                nc.gpsimd.iota(pos_f[:, lo:hi], pattern=[[1, hi - lo]], base=lo,
                               channel_multiplier=0,
                               allow_small_or_imprecise_dtypes=True)
            ym_sb = consts.tile([CH, BPC * LCN], f32)
            # W carries DerivErf's 2/sqrt(pi) factor; scaling eps by the same
            # factor makes rd = ym/(k*sumW + k*eps) = ym/k/(sumW + eps) exact.
            eps_sb = consts.tile([CH, 1], f32)
            nc.vector.memset(eps_sb, float(EPS) * 2.0 / np.pi ** 0.5)
            # warm the ACT spline tables during the input DMAs
            tblw = consts.tile([CH, 1], f32)
            nc.scalar.activation(out=tblw, in_=eps_sb, func=FT.Derivative_Erf)

            def col(tile_, idx):
                return tile_[:, idx:idx + 1]

            def cidx(q, b, t):
                return (q * BPC + b) * TCN + t

            xta_tiles = {}
            w_tiles = {}

            def load_xta(b):
                xta_sb = xta_p.tile([CH, TCN, C + 1], bf16)
                nc.sync.dma_start(out=xta_sb, in_=xta_d[b])
                if not trivial_masks:
                    for t in range(TCN):
                        # x_mask fold on GpSimd (broadcast mult, x cols only)
                        nc.gpsimd.tensor_tensor(
                            out=xta_sb[:, t, :C], in0=xta_sb[:, t, :C],
                            in1=bcast(col(coefs_sb, cidx(2, b, t)), C),
                            op=OP.mult,
                        )
                xta_tiles[b] = xta_sb

            def wgen(b, t, halves=1):
                bs, be = bands[b][t]
                bw = be - bs
                # mu on DVE only: concurrent GpSimd streaming steals the DVE's
                # second SBUF port and drops tensor_scalar from 2x to 1x.
                wt = w_pools[t].tile([CH, wmax], bf16)
                step = (bw // halves + CH - 1) // CH * CH
                for lo in range(0, bw, step):
                    hi = min(bw, lo + step)
                    mu = mu_p.tile([CH, wmax], f32, tag="mu")
                    if b == 0 and t == 0 and trivial_masks:
                        # first W tile: mu on ScalarE (Identity + negated
                        # center bias) — it never contends with the iotas
                        # still streaming on GpSimd, unlike DVE's 2x mode
                        nc.scalar.activation(
                            out=mu[:, :hi - lo], in_=pos_f[:, bs + lo:bs + hi],
                            func=FT.Identity,
                            bias=col(coefs_sb, cidx(2, b, t)),
                        )
                    else:
                        nc.vector.tensor_scalar(
                            out=mu[:, :hi - lo], in0=pos_f[:, bs + lo:bs + hi],
                            scalar1=col(coefs_sb, cidx(0, b, t)), scalar2=None,
                            op0=OP.subtract,
                        )
                    # W = 2/sqrt(pi)*exp(-(s*mu)^2); constant cancels via rd
                    nc.scalar.activation(
                        out=wt[:, lo:hi], in_=mu[:, :hi - lo],
                        func=FT.Derivative_Erf,
                        scale=col(coefs_sb, cidx(1, b, t)),
                    )
                w_tiles[(b, t)] = wt

            ogrp_live = {}

            def group(b, g):
                sb = bands[b]
                pgrp = psum_p.tile([CH, GRP, 512], f32, tag="pgrp")
                for k in range(GRP):
                    j = g * GRP + k
                    lo = j * CH
                    ctc = [t for t in range(TCN) if sb[t][0] <= lo and lo + CH <= sb[t][1]]
                    if not ctc:
                        nc.vector.memset(pgrp[:, k, :C + 1], 0.0)
                        continue
                    for i, t in enumerate(ctc):
                        off = lo - sb[t][0]
                        nc.tensor.matmul(
                            out=pgrp[:, k, :C + 1],
                            lhsT=w_tiles[(b, t)][:, off:off + CH],
                            rhs=xta_tiles[b][:, t, :],
                            start=(i == 0), stop=(i == len(ctc) - 1),
                        )
                dtmp = small_p.tile([CH, GRP], f32, tag="dtmp")
                # d + eps: on ACT for DVE-normed groups (stage pipelining —
                # ACT extracts d while DVE finishes the previous group's
                # norm), on DVE for ACT-normed groups.
                if (b * NGRP + g) % 4 < 3:
                    nc.scalar.activation(out=dtmp, in_=pgrp[:, :, C],
                                         func=FT.Identity, bias=eps_sb[:, 0:1])
                else:
                    nc.vector.tensor_scalar(
                        out=dtmp, in0=pgrp[:, :, C],
                        scalar1=float(EPS) * 1.1283791670955126,
                        scalar2=None, op0=OP.add,
                    )
                rd = small_p.tile([CH, GRP], f32, tag="rd")
                nc.vector.reciprocal(out=rd, in_=dtmp)
                if not trivial_masks:
                    nc.gpsimd.tensor_tensor(
                        out=rd, in0=rd,
                        in1=ym_sb[:, b * LCN + g * GRP: b * LCN + g * GRP + GRP],
                        op=OP.mult,
                    )
                # ogrp spans a PAIR of psum groups -> one out-DMA per pair
                half = g % 2
                if half == 0:
                    ogrp_new = out_p.tile([CH, 2 * GRP, C], f32, tag="ogrp")
                    ogrp_live[b] = ogrp_new
                ogrp = ogrp_live[b]
                osl = ogrp[:, half * GRP:(half + 1) * GRP, :]
                tail = (b == 1 and g >= NGRP - 4)
                if tail:
                    # drain phase: split each group across DVE+ACT in parallel
                    # to shorten the trailing chain latency
                    nc.vector.tensor_scalar_mul(
                        out=osl[:, 0, :], in0=pgrp[:, 0, :C], scalar1=col(rd, 0))
                    nc.scalar.activation(
                        out=osl[:, 1, :], in_=pgrp[:, 1, :C],
                        func=FT.Copy, scale=col(rd, 1))
                elif (b * NGRP + g) % 4 < 3:
                    # normalize all chunks in one DVE op (rd broadcast on a
                    # stride-0 free dim)
                    rdb = bass.AP(tensor=rd.tensor, offset=rd.offset,
                                  ap=[rd.ap[0], rd.ap[1], [0, C]])
                    nc.vector.tensor_tensor(
                        out=osl, in0=pgrp[:, :, :C], in1=rdb, op=OP.mult,
                    )
                else:
                    for k in range(GRP):
                        nc.scalar.activation(
                            out=osl[:, k, :], in_=pgrp[:, k, :C],
                            func=FT.Copy, scale=col(rd, k),
                        )
                if b == 1 and g >= NGRP - 2:
                    # final stretch: per-group DMAs on sync+scalar (never
                    # gpsimd: the SWDGE drain at kernel exit costs ~5us if
                    # POOL still has DMA work queued at the end)
                    eng = nc.sync if g % 2 == 0 else nc.scalar
                    eng.dma_start(
                        out=out_d[b, g * GRP * CH:(g + 1) * GRP * CH, :]
                        .rearrange("(k p) c -> p k c", p=CH),
                        in_=osl,
                    )
                elif half == 1:
                    # all output DMAs on the sync HWDGE ring: any SWDGE
                    # (gpsimd) DMA anywhere makes the kernel-exit POOL drain
                    # cost ~5us of queue quiesce
                    pair = g // 2
                    nc.sync.dma_start(
                        out=out_d[b, pair * 2 * GRP * CH:(pair + 1) * 2 * GRP * CH, :]
                        .rearrange("(k p) c -> p k c", p=CH),
                        in_=ogrp,
                    )

            # batch 0 weight phase, then interleave batch 1's weight phase
            # into batch 0's matmul/normalize groups to keep all engines fed.
            # Emission order matters: each wgen's DMA-lane wait covers every
            # DMA emitted before it, so DMAs are interleaved to need-time.
            wgen(0, 0, halves=2)   # needs iota1 + coefs only; first half
            load_xta(0)            # unblocks the warm-up matmuls early
            wgen(0, 1)
            # PE warm-up on real data: back-to-back matmuls bridge the gap
            # until the group stream starts, so HAM un-throttles and the
            # real matmuls run at 2.4GHz.
            wps = psum_p.tile([CH, GRP, 512], f32, tag="pgrp")
            for i in range(12):
                nc.tensor.matmul(
                    out=wps[:, 0, :C + 1], lhsT=w_tiles[(0, 0)][:, :CH],
                    rhs=xta_tiles[0][:, 0, :], start=True, stop=True,
                )
            if not trivial_masks:
                nc.sync.dma_start(out=ym_sb, in_=ym_d[:, :])
            # last pos piece deferred: only tc3's W needs it, and running it
            # earlier steals the DVE port from mu(0,1) (2x -> 1x)
            nc.gpsimd.iota(pos_f[:, IW2:], pattern=[[1, L - IW2]], base=IW2,
                           channel_multiplier=0,
                           allow_small_or_imprecise_dtypes=True)
            for t in range(2, TCN):
                wgen(0, t)
            load_xta(1)
            for g in range(NGRP // 2):
                group(0, g)
                if g in (1, 3, 5, 7):
                    wgen(1, g // 2)
            # interleave the second half of batch 0 with the start of batch 1
            # to flatten the transition and spread the normalize load
            for g in range(NGRP // 2, NGRP):
                group(0, g)
                group(1, g - NGRP // 2)
            for g in range(NGRP // 2, NGRP):
                group(1, g)
    return nc


def _prepare_inputs(x, w, x_mask, y_mask, sigma_scale):
    center, s = _center_scale(w, sigma_scale[0])
    bands = _bands(center, w)

    xt = np.ascontiguousarray(x.transpose(0, 2, 1))          # (B, T, C)
    xta = np.concatenate([xt, np.ones((B, T, 1), np.float32)], axis=2)
    # device layout [b, p, tc, c] for a descriptor-light DMA
    xta = np.ascontiguousarray(
        xta.reshape(B, TCN, CH, C + 1).transpose(0, 2, 1, 3)).astype(_bf16)

    xm = np.broadcast_to(x_mask.reshape(B, T), (B, T)).astype(np.float32)
    ymf = np.broadcast_to(y_mask.reshape(B, L), (B, L)).astype(np.float32)
    trivial_masks = bool(np.all(xm == 1.0) and np.all(ymf == 1.0))

    in_maps = []
    for core in range(N_CORES):
        bsel = [core * BPC + s_ for s_ in range(BPC)]
        coefs = np.empty((3, BPC, TCN, CH), np.float32)
        for s_, bb in enumerate(bsel):
            coefs[0, s_] = center[bb].reshape(TCN, CH)
            coefs[1, s_] = s[bb].reshape(TCN, CH)
            # row 2: x_mask when masks are active, else -center (bias form
            # for the ScalarE Identity mu path)
            coefs[2, s_] = (xm[bb] if not trivial_masks else -center[bb]).reshape(TCN, CH)
        ym_c = np.stack([ymf[bb].reshape(LCN, CH) for bb in bsel])  # (BPC,LCN,CH)
        in_maps.append({
            "xta": xta[bsel],
            "coefs": np.ascontiguousarray(
                coefs.reshape(3 * BPC * TCN, CH).T),          # [CH, 24]
            "ym": np.ascontiguousarray(
                ym_c.reshape(BPC * LCN, CH).T),               # [CH, 64]
        })
    band_key = (tuple(tuple(tuple(p) for p in sb) for sb in bands),
                trivial_masks)
    return in_maps, band_key


def kernel(x, w, x_mask, y_mask, sigma_scale):
    x = np.asarray(x, dtype=np.float32)
    w = np.asarray(w, dtype=np.float32)
    x_mask = np.asarray(x_mask, dtype=np.float32)
    y_mask = np.asarray(y_mask, dtype=np.float32)
    sigma_scale = np.asarray(sigma_scale, dtype=np.float32)
    assert x.shape == (B, C, T) and w.shape == (B, T)

    in_maps, band_key = _prepare_inputs(x, w, x_mask, y_mask, sigma_scale)

    if band_key not in _cache:
        nc = _build(band_key)
        _split_excess_waits(nc)
        _cache[band_key] = nc
    nc = _cache[band_key]

    from concourse.bass_utils import run_bass_kernel_spmd

    res = run_bass_kernel_spmd(nc, in_maps, list(range(N_CORES)), trace=False)
    outs = [res.results[i]["out"] for i in range(N_CORES)]      # (BPC, L, C) each
    full = np.concatenate(outs, axis=0)                          # (B, L, C)
    return full.transpose(0, 2, 1)                               # (B, C, L)



# revision 2
# speedup vs baseline: 1.1221x; 1.1221x over previous
"""DifferentiableLengthRegulator Trainium2 kernel.

out[b,c,l] = y_mask * (sum_t x[b,c,t]*W[b,t,l]) / (sum_t W[b,t,l] + eps)
W = exp(-0.5*(l - center[b,t])^2 / (w[b,t]^2*sigma_scale^2 + eps))

Sharding: data-parallel over batch B=16 -> 8 cores x 2 batches.
Per core, per batch (banded over the frame axis; Gaussian weights vanish
outside ~5 sigma of each token chunk's centers):
  ACT : W = DerivErf(s*pos + (-s*center)) -> bf16  (scale+bias per-partition
        fold: the mu = pos - center stage is computed inside the ACT op,
        freeing the DVE entirely; DerivErf's 2/sqrt(pi) cancels via rd)
  PE  : psum[l, 0:257] += W_tc[:, lslice]^T @ [xT | ones]   (bf16, 4 frame
        chunks per 4-bank psum tile, double buffered)
  DVE : d+eps = tensor_scalar(psum col 256); rd = 1/(d+eps);
        evac psum*rd -> bf16 out tile (ACT takes some chunks as Copy-scale
        to balance engine load)
Output written (B, L, C) bf16; host converts to fp32 + transposes; x_mask /
y_mask folded on host (exact for the all-ones case and cheap otherwise).
"""

import numpy as np
import ml_dtypes

B, C, T, L = 16, 256, 512, 4096
N_CORES = 8
BPC = B // N_CORES  # batches per core
CH = 128            # partition chunk
TCN = T // CH       # 4 token chunks
LCN = L // CH       # 32 frame chunks
GRP = 4             # frame chunks per psum tile (4 banks)
NT = LCN // GRP     # 8 tiles per batch
EPS = 1e-8
K_DERF = 1.1283791670955126  # 2/sqrt(pi), DerivErf's constant factor
MARGIN_SIGMA = 5.0
BAND_ALIGN = 128
NWARM = 12

_bf16 = ml_dtypes.bfloat16
_cache = {}

# DVE evac chunks per tile (out of GRP=4); remainder goes to ACT as
# Copy-scale. b0's odd tiles run while ACT also generates b1's W tiles.
ND_PATTERN = {
    0: [2, 3, 3, 3, 2, 3, 3, 3],
    1: [3, 2, 3, 2, 3, 2, 3, 2],
}


def _center_scale(w, sigma_scale):
    """Mirror the reference's cumsum/center math (same jax backend bits)."""
    try:
        import jax.numpy as jnp

        wj = jnp.asarray(w)
        center = np.asarray(jnp.cumsum(wj, axis=1) - 0.5 * wj, dtype=np.float32)
    except Exception:
        center = (np.cumsum(w, axis=1, dtype=np.float32) - 0.5 * w).astype(np.float32)
    sigma = (w * np.float32(sigma_scale)).astype(np.float32)
    # W = DerivErf(s*mu)*sqrt(pi)/2 = exp(-(s*mu)^2), s = sqrt(0.5/(sig^2+eps))
    s = np.sqrt(np.float32(0.5) / (np.square(sigma) + np.float32(EPS))).astype(np.float32)
    return center, s


def _bands(center, w_all):
    """Per (slot, tc) aligned frame band, unioned across cores (SPMD)."""
    bands = []
    for slot in range(BPC):
        rows = center[slot::BPC]      # the 8 batches that land in this slot
        wrows = w_all[slot::BPC]
        sb = []
        for tc in range(TCN):
            seg = rows[:, tc * CH:(tc + 1) * CH]
            margin = float(MARGIN_SIGMA * wrows[:, tc * CH:(tc + 1) * CH].max() + 1.0)
            bs = max(0, int(np.floor((seg.min() - margin) / BAND_ALIGN)) * BAND_ALIGN)
            be = min(L, int(np.ceil((seg.max() + margin) / BAND_ALIGN)) * BAND_ALIGN)
            if tc == 0:
                bs = 0
            if tc == TCN - 1:
                be = L
            bs = min(bs, be - CH)
            sb.append((bs, be))
        bands.append(sb)
    # every frame chunk must be covered by at least one token chunk's band
    for sb in bands:
        for chunk in range(LCN):
            lo = chunk * CH
            assert any(bs <= lo and lo + CH <= be for bs, be in sb), (
                f"frame chunk {chunk} uncovered; widen MARGIN_SIGMA"
            )
    return bands


def _split_excess_waits(nc, max_waits=1):
    """walrus here caps sync-waits at 1 per compute instruction; move the
    excess onto injected same-engine NoOps just before the instruction
    (waiting earlier on the same engine is always safe)."""
    from concourse import mybir

    for f in nc.m.functions:
        for blk in f.blocks:
            new = []
            for inst in blk.instructions:
                si = inst.sync_info
                if si is not None and len(si.on_wait) > max_waits:
                    waits = list(si.on_wait)
                    keep, extra = waits[-max_waits:], waits[:-max_waits]
                    for i in range(0, len(extra), max_waits):
                        nop = mybir.InstNoOp(name=f"{inst.name}-xw{i}", ins=[], outs=[])
                        nop.engine = inst.engine
                        nop.sync_info = mybir.SyncInfo(
                            on_wait=extra[i:i + max_waits], on_update=[])
                        new.append(nop)
                    inst.sync_info = mybir.SyncInfo(
                        on_wait=keep, on_update=list(si.on_update))
                new.append(inst)
            blk.instructions = new


def _slim_tile_exit(tile):
    """Drop the second all-engine barrier in Tile's exit sequence: the
    sem-clears it orders are already completed by each engine finishing its
    own instruction stream before the NEFF ends (~4us saved)."""
    if getattr(tile.TileContext, "_slim_exit", False):
        return
    ScopedClock = tile.ScopedClock

    def _drain_and_barrier(self, tick_clock, wait_clock):
        drain_inst = self.nc.sync.drain()
        wait_clock.add_sem_waits(
            drain_inst.ins, ScopedClock({None: tick_clock.global_clock}))
        self.nc.all_engine_barrier()
        popped = self.nc._tile_sem_poison_stack.pop()
        assert popped is self._sem_poison
        self.nc.clear_and_free_semaphores(list(self.sems.allocated().values()))

    tile.TileContext._drain_and_barrier = _drain_and_barrier
    tile.TileContext._slim_exit = True


def _build(band_key):
    import concourse.bass as bass
    import concourse.tile as tile
    from concourse import mybir

    _slim_tile_exit(tile)
    bands = [[(band_key[s][t][0], band_key[s][t][1]) for t in range(TCN)]
             for s in range(BPC)]
    wmax = [max(bands[s][t][1] - bands[s][t][0] for s in range(BPC))
            for t in range(TCN)]

    nc = bass.Bass("TRN2", target_bir_lowering=False, debug=False)
    # xta host layout: [b, p, tc, c] so the DMA is descriptor-light
    xta_d = nc.declare_dram_parameter("xta", [BPC, CH, TCN, C + 1], mybir.dt.bfloat16, isOutput=False)
    coefs_d = nc.declare_dram_parameter("coefs", [CH, 2 * BPC * TCN], mybir.dt.float32, isOutput=False)
    out_d = nc.declare_dram_parameter("out", [BPC, L, C], mybir.dt.bfloat16, isOutput=True)

    f32 = mybir.dt.float32
    bf16 = mybir.dt.bfloat16
    FT = mybir.ActivationFunctionType
    OP = mybir.AluOpType
    EPS_K = float(EPS) * K_DERF

    with tile.TileContext(nc) as tc_:
        import contextlib

        with contextlib.ExitStack() as ctx:
            consts = ctx.enter_context(tc_.tile_pool(name="consts", bufs=1))
            xta_p = ctx.enter_context(tc_.tile_pool(name="xta", bufs=2))
            w_pools = [ctx.enter_context(tc_.tile_pool(name=f"w{t}", bufs=2)) for t in range(TCN)]
            psum_p = ctx.enter_context(tc_.tile_pool(name="ps", bufs=2, space="PSUM"))
            small_p = ctx.enter_context(tc_.tile_pool(name="small", bufs=6))
            out_p = ctx.enter_context(tc_.tile_pool(name="osb", bufs=4))

            def col(tile_, idx):
                return tile_[:, idx:idx + 1]

            def cidx(q, b, t):
                # q=0 -> s (scale), q=1 -> -s*center (bias)
                return (q * BPC + b) * TCN + t

            # --- constants. pos comes entirely from iota on GpSimd (no DMA
            # wait at all); coefs on the sync HWDGE ring.
            coefs_sb = consts.tile([CH, 2 * BPC * TCN], f32)
            nc.sync.dma_start(out=coefs_sb, in_=coefs_d[:, :])
            # warm the ACT spline tables during the input DMAs
            warm = consts.tile([CH, 1], f32)
            nc.vector.memset(warm, 0.0)
            nc.scalar.activation(out=warm, in_=warm, func=FT.Derivative_Erf)

            # pos iota pieces (GpSimd runs them back-to-back from t=0);
            # piece boundaries chosen so W(0,0)'s first half only waits on
            # piece 1.
            pos_f = consts.tile([CH, L], f32)
            IW = max(bands[s][0][1] for s in range(BPC))
            IW2 = max(bands[s][1][1] for s in range(BPC))
            IH = (IW // 2 + CH - 1) // CH * CH
            for lo, hi in ((0, IH), (IH, IW), (IW, IW2), (IW2, L)):
                if hi > lo:
                    nc.gpsimd.iota(pos_f[:, lo:hi], pattern=[[1, hi - lo]], base=lo,
                                   channel_multiplier=0,
                                   allow_small_or_imprecise_dtypes=True)

            xta_tiles = {}
            w_tiles = {}

            def load_xta(b):
                xta_sb = xta_p.tile([CH, TCN, C + 1], bf16)
                nc.sync.dma_start(out=xta_sb, in_=xta_d[b])
                xta_tiles[b] = xta_sb

            def wgen(b, t, split_at=None):
                bs, be = bands[b][t]
                wt = w_pools[t].tile([CH, wmax[t]], bf16)
                cuts = [bs, be] if split_at is None else [bs, split_at, be]
                for lo, hi in zip(cuts[:-1], cuts[1:]):
                    # W = 2/sqrt(pi)*exp(-(s*pos - s*center)^2); the constant
                    # cancels via rd, the bias fold removes the mu stage.
                    nc.scalar.activation(
                        out=wt[:, lo - bs:hi - bs], in_=pos_f[:, lo:hi],
                        func=FT.Derivative_Erf,
                        scale=col(coefs_sb, cidx(0, b, t)),
                        bias=col(coefs_sb, cidx(1, b, t)),
                    )
                w_tiles[(b, t)] = wt

            def tile_ops(b, ti, tail=False):
                sb = bands[b]
                pgrp = psum_p.tile([CH, GRP, 512], f32, tag="pgrp")
                for k in range(GRP):
                    lo = (ti * GRP + k) * CH
                    ctc = [t for t in range(TCN) if sb[t][0] <= lo and lo + CH <= sb[t][1]]
                    for i, t in enumerate(ctc):
                        off = lo - sb[t][0]
                        nc.tensor.matmul(
                            out=pgrp[:, k, :C + 1],
                            lhsT=w_tiles[(b, t)][:, off:off + CH],
                            rhs=xta_tiles[b][:, t, :],
                            start=(i == 0), stop=(i == len(ctc) - 1),
                        )
                dtmp = small_p.tile([CH, GRP], f32, tag="dtmp")
                nc.vector.tensor_scalar(
                    out=dtmp, in0=pgrp[:, :, C],
                    scalar1=EPS_K, scalar2=None, op0=OP.add,
                )
                rd = small_p.tile([CH, GRP], f32, tag="rd")
                nc.vector.reciprocal(out=rd, in_=dtmp)
                osb = out_p.tile([CH, GRP, C], bf16, tag="osb")
                if tail:
                    # drain phase: DVE and ACT evac chunk pairs in parallel,
                    # two half DMAs on independent queues
                    rdb = bass.AP(tensor=rd.tensor, offset=rd.offset,
                                  ap=[rd.ap[0], [rd.ap[1][0], 2], [0, C]])
                    nc.vector.tensor_tensor(
                        out=osb[:, :2, :], in0=pgrp[:, :2, :C], in1=rdb, op=OP.mult)
                    for k in (2, 3):
                        nc.scalar.activation(
                            out=osb[:, k, :], in_=pgrp[:, k, :C],
                            func=FT.Copy, scale=col(rd, k))
                    base = ti * GRP * CH
                    nc.sync.dma_start(
                        out=out_d[b, base:base + 2 * CH, :]
                        .rearrange("(k p) c -> p k c", p=CH),
                        in_=osb[:, :2, :])
                    nc.scalar.dma_start(
                        out=out_d[b, base + 2 * CH:base + 4 * CH, :]
                        .rearrange("(k p) c -> p k c", p=CH),
                        in_=osb[:, 2:, :])
                    return
                nd = ND_PATTERN[b][ti]
                rdb = bass.AP(tensor=rd.tensor, offset=rd.offset,
                              ap=[rd.ap[0], [rd.ap[1][0], nd], [0, C]])
                nc.vector.tensor_tensor(
                    out=osb[:, :nd, :], in0=pgrp[:, :nd, :C], in1=rdb, op=OP.mult)
                for k in range(nd, GRP):
                    nc.scalar.activation(
                        out=osb[:, k, :], in_=pgrp[:, k, :C],
                        func=FT.Copy, scale=col(rd, k))
                nc.sync.dma_start(
                    out=out_d[b, ti * GRP * CH:(ti + 1) * GRP * CH, :]
                    .rearrange("(k p) c -> p k c", p=CH),
                    in_=osb)

            # --- schedule. Emission order per engine = execution order.
            wgen(0, 0, split_at=min(IH, bands[0][0][1]))
            load_xta(0)
            wgen(0, 1)
            # PE warm-up on real data: back-to-back matmuls bridge the gap
            # until the tile stream starts so the clock ramps to 2.4GHz.
            wps = psum_p.tile([CH, GRP, 512], f32, tag="pgrp")
            for _ in range(NWARM):
                nc.tensor.matmul(
                    out=wps[:, 0, :C + 1], lhsT=w_tiles[(0, 0)][:, :CH],
                    rhs=xta_tiles[0][:, 0, :], start=True, stop=True,
                )
            wgen(0, 2)
            wgen(0, 3)
            load_xta(1)
            for ti in range(NT):
                tile_ops(0, ti)
                if ti in (1, 3, 5, 7):
                    wgen(1, ti // 2)
            for ti in range(NT):
                tile_ops(1, ti, tail=(ti == NT - 1))
    return nc


def _prepare_inputs(x, w, x_mask, y_mask, sigma_scale):
    center, s = _center_scale(w, sigma_scale[0])
    bands = _bands(center, w)
    nb = (-(s * center)).astype(np.float32)    # bias: -s*center (one f32 round)

    xm = np.broadcast_to(x_mask.reshape(B, T), (B, T)).astype(np.float32)
    if not np.all(xm == 1.0):
        x = (x * xm[:, None, :]).astype(np.float32)

    xt = np.ascontiguousarray(x.transpose(0, 2, 1))          # (B, T, C)
    xta = np.concatenate([xt, np.ones((B, T, 1), np.float32)], axis=2)
    # device layout [b, p, tc, c] for a descriptor-light DMA
    xta = np.ascontiguousarray(
        xta.reshape(B, TCN, CH, C + 1).transpose(0, 2, 1, 3)).astype(_bf16)

    in_maps = []
    for core in range(N_CORES):
        bsel = [core * BPC + s_ for s_ in range(BPC)]
        coefs = np.empty((2, BPC, TCN, CH), np.float32)
        for s_, bb in enumerate(bsel):
            coefs[0, s_] = s[bb].reshape(TCN, CH)
            coefs[1, s_] = nb[bb].reshape(TCN, CH)
        in_maps.append({
            "xta": xta[bsel],
            "coefs": np.ascontiguousarray(
                coefs.reshape(2 * BPC * TCN, CH).T),          # [CH, 16]
        })
    band_key = tuple(tuple(tuple(p) for p in sb) for sb in bands)
    return in_maps, band_key


def kernel(x, w, x_mask, y_mask, sigma_scale):
    x = np.asarray(x, dtype=np.float32)
    w = np.asarray(w, dtype=np.float32)
    x_mask = np.asarray(x_mask, dtype=np.float32)
    y_mask = np.asarray(y_mask, dtype=np.float32)
    sigma_scale = np.asarray(sigma_scale, dtype=np.float32)
    assert x.shape == (B, C, T) and w.shape == (B, T)

    in_maps, band_key = _prepare_inputs(x, w, x_mask, y_mask, sigma_scale)

    if band_key not in _cache:
        nc = _build(band_key)
        _split_excess_waits(nc)
        _cache[band_key] = nc
    nc = _cache[band_key]

    from concourse.bass_utils import run_bass_kernel_spmd

    res = run_bass_kernel_spmd(nc, in_maps, list(range(N_CORES)), trace=False)
    outs = [np.asarray(res.results[i]["out"]).astype(np.float32)
            for i in range(N_CORES)]                             # (BPC, L, C)
    full = np.concatenate(outs, axis=0).transpose(0, 2, 1)       # (B, C, L)
    ym = np.broadcast_to(y_mask.reshape(B, L), (B, L)).astype(np.float32)
    if not np.all(ym == 1.0):
        full = full * ym[:, None, :]
    return full


# revision 5
# speedup vs baseline: 1.4206x; 1.2660x over previous
"""DifferentiableLengthRegulator Trainium2 kernel.

out[b,c,l] = y_mask * (sum_t x[b,c,t]*W[b,t,l]) / (sum_t W[b,t,l] + eps)
W = exp(-0.5*(l - center[b,t])^2 / (w[b,t]^2*sigma_scale^2 + eps))

Sharding: data-parallel over batch B=16 -> 8 cores x 2 batches.
Per core, per batch (banded over the frame axis; Gaussian weights vanish
outside ~5 sigma of each token chunk's centers):
  ACT : W = DerivErf(s*pos + (-s*center)) -> bf16  (per-partition scale+bias
        fold computes mu inside the ACT op: no DVE mu stage at all)
  PE  : psum[l, 0:257] += W_tc[:, lslice]^T @ [xT | ones]
  DVE : d+eps = tensor_scalar(psum cols 256); rd = 1/(d+eps);
        evac psum*rd -> bf16 (ACT takes some chunks as Copy-scale)
PSUM is one manually-rotated [CH, 8, 512] arena (8 banks): chunk j lives in
slot j%8, giving depth-8 rotation with per-range dependencies instead of
depth-2 tile-pool rotation (PE never waits on evac).
Output layout [BPC, NT, CH, GRP, C] bf16 -> 2KB contiguous DMA lines; host
reshapes to (B, C, L) fp32. x_mask / y_mask folded on host.
"""

import numpy as np
import ml_dtypes

B, C, T, L = 16, 256, 512, 4096
N_CORES = 8
BPC = B // N_CORES  # batches per core
CH = 128            # partition chunk
TCN = T // CH       # 4 token chunks
LCN = L // CH       # 32 frame chunks
GRP = 4             # frame chunks per evac group
NT = LCN // GRP     # 8 groups per batch
NSLOT = 8           # psum arena slots (banks)
EPS = 1e-8
K_DERF = 1.1283791670955126  # 2/sqrt(pi), DerivErf's constant factor
MARGIN_SIGMA = 5.0
BAND_ALIGN = 128
NWARM = 12

_bf16 = ml_dtypes.bfloat16
_cache = {}

# DVE evac chunks per group (out of GRP=4); remainder goes to ACT as
# Copy-scale. b0's groups run while ACT also generates later W tiles.
ND_PATTERN = {0: [3] * 8, 1: [2] * 8}


def _center_scale(w, sigma_scale):
    """Mirror the reference's cumsum/center math (same jax backend bits)."""
    try:
        import jax.numpy as jnp

        wj = jnp.asarray(w)
        center = np.asarray(jnp.cumsum(wj, axis=1) - 0.5 * wj, dtype=np.float32)
    except Exception:
        center = (np.cumsum(w, axis=1, dtype=np.float32) - 0.5 * w).astype(np.float32)
    sigma = (w * np.float32(sigma_scale)).astype(np.float32)
    # W = DerivErf(s*mu)*sqrt(pi)/2 = exp(-(s*mu)^2), s = sqrt(0.5/(sig^2+eps))
    s = np.sqrt(np.float32(0.5) / (np.square(sigma) + np.float32(EPS))).astype(np.float32)
    return center, s


def _bands(center, w_all):
    """Per (slot, tc) aligned frame band, unioned across cores (SPMD)."""
    bands = []
    for slot in range(BPC):
        rows = center[slot::BPC]      # the 8 batches that land in this slot
        wrows = w_all[slot::BPC]
        sb = []
        for tc in range(TCN):
            seg = rows[:, tc * CH:(tc + 1) * CH]
            margin = float(MARGIN_SIGMA * wrows[:, tc * CH:(tc + 1) * CH].max() + 1.0)
            bs = max(0, int(np.floor((seg.min() - margin) / BAND_ALIGN)) * BAND_ALIGN)
            be = min(L, int(np.ceil((seg.max() + margin) / BAND_ALIGN)) * BAND_ALIGN)
            if tc == 0:
                bs = 0
            if tc == TCN - 1:
                be = L
            bs = min(bs, be - CH)
            sb.append((bs, be))
        bands.append(sb)
    for sb in bands:
        for chunk in range(LCN):
            lo = chunk * CH
            assert any(bs <= lo and lo + CH <= be for bs, be in sb), (
                f"frame chunk {chunk} uncovered; widen MARGIN_SIGMA"
            )
    return bands


def _split_excess_waits(nc, max_waits=1):
    """walrus here caps sync-waits at 1 per compute instruction; move the
    excess onto injected same-engine NoOps just before the instruction
    (waiting earlier on the same engine is always safe)."""
    from concourse import mybir

    for f in nc.m.functions:
        for blk in f.blocks:
            new = []
            for inst in blk.instructions:
                si = inst.sync_info
                if si is not None and len(si.on_wait) > max_waits:
                    waits = list(si.on_wait)
                    keep, extra = waits[-max_waits:], waits[:-max_waits]
                    for i in range(0, len(extra), max_waits):
                        nop = mybir.InstNoOp(name=f"{inst.name}-xw{i}", ins=[], outs=[])
                        nop.engine = inst.engine
                        nop.sync_info = mybir.SyncInfo(
                            on_wait=extra[i:i + max_waits], on_update=[])
                        new.append(nop)
                    inst.sync_info = mybir.SyncInfo(
                        on_wait=keep, on_update=list(si.on_update))
                new.append(inst)
            blk.instructions = new


def _slim_tile_exit(tile):
    """Drop the second all-engine barrier in Tile's exit sequence: the
    sem-clears it orders are already completed by each engine finishing its
    own instruction stream before the NEFF ends (~4us saved)."""
    if getattr(tile.TileContext, "_slim_exit", False):
        return
    ScopedClock = tile.ScopedClock

    def _drain_and_barrier(self, tick_clock, wait_clock):
        drain_inst = self.nc.sync.drain()
        wait_clock.add_sem_waits(
            drain_inst.ins, ScopedClock({None: tick_clock.global_clock}))
        self.nc.all_engine_barrier()
        popped = self.nc._tile_sem_poison_stack.pop()
        assert popped is self._sem_poison
        self.nc.clear_and_free_semaphores(list(self.sems.allocated().values()))

    tile.TileContext._drain_and_barrier = _drain_and_barrier
    tile.TileContext._slim_exit = True


def _build(band_key):
    import concourse.bass as bass
    import concourse.tile as tile
    from concourse import mybir

    _slim_tile_exit(tile)
    bands = [[(band_key[s][t][0], band_key[s][t][1]) for t in range(TCN)]
             for s in range(BPC)]
    wmax = [max(bands[s][t][1] - bands[s][t][0] for s in range(BPC))
            for t in range(TCN)]

    nc = bass.Bass("TRN2", target_bir_lowering=False, debug=False)
    # xta host layout: [b, p, tc, c] so the DMA is descriptor-light
    xta_d = nc.declare_dram_parameter("xta", [BPC, CH, TCN, C + 1], mybir.dt.bfloat16, isOutput=False)
    coefs_d = nc.declare_dram_parameter("coefs", [CH, 2 * BPC * TCN], mybir.dt.float32, isOutput=False)
    out_d = nc.declare_dram_parameter("out", [BPC, NT, CH, GRP, C], mybir.dt.bfloat16, isOutput=True)

    f32 = mybir.dt.float32
    bf16 = mybir.dt.bfloat16
    FT = mybir.ActivationFunctionType
    OP = mybir.AluOpType
    EPS_K = float(EPS) * K_DERF

    # first group (in the b0..b1 stream) whose matmuls need W(b, t)
    def first_need(b, t):
        bs = bands[b][t][0]
        return b * NT + bs // (GRP * CH)

    with tile.TileContext(nc) as tc_:
        import contextlib

        with contextlib.ExitStack() as ctx:
            consts = ctx.enter_context(tc_.tile_pool(name="consts", bufs=1))
            xta_p = ctx.enter_context(tc_.tile_pool(name="xta", bufs=2))
            w_pools = [ctx.enter_context(tc_.tile_pool(name=f"w{t}", bufs=2)) for t in range(TCN)]
            psum_p = ctx.enter_context(tc_.tile_pool(name="ps", bufs=1, space="PSUM"))
            small_p = ctx.enter_context(tc_.tile_pool(name="small", bufs=6))
            out_p = ctx.enter_context(tc_.tile_pool(name="osb", bufs=4))

            def col(tile_, idx):
                return tile_[:, idx:idx + 1]

            def cidx(q, b, t):
                # q=0 -> s (scale), q=1 -> -s*center (bias)
                return (q * BPC + b) * TCN + t

            xta_tiles = {}

            def load_xta_tc(b, t):
                if b not in xta_tiles:
                    xta_sb = xta_p.tile([CH, TCN, C + 1], bf16, tag="xta")
                    xta_tiles[b] = xta_sb
                nc.sync.dma_start(out=xta_tiles[b][:, t, :], in_=xta_d[b, :, t, :])

            # --- startup: first xta piece, coefs, ACT table warm, pos build.
            load_xta_tc(0, 0)
            coefs_sb = consts.tile([CH, 2 * BPC * TCN], f32)
            nc.sync.dma_start(out=coefs_sb, in_=coefs_d[:, :])
            for t in range(1, TCN):
                load_xta_tc(0, t)

            warm = consts.tile([CH, 1], f32)
            nc.vector.memset(warm, 0.0)
            nc.scalar.activation(out=warm, in_=warm, func=FT.Derivative_Erf)

            # pos[l] = l, built cooperatively: GpSimd iota for the head and
            # tail pieces, DVE coarse+fine composed adds for the middle
            # (GpSimd iota is 1.8ns/col; DVE is 1.04 and idle before the
            # group stream starts).
            pos_f = consts.tile([CH, L], f32)
            fine = consts.tile([CH, CH], f32)
            coarse = consts.tile([CH, LCN], f32)
            P1 = 256
            IW = max(bands[s][0][1] for s in range(BPC))
            IW2 = max(bands[s][1][1] for s in range(BPC))
            nc.gpsimd.iota(fine, pattern=[[1, CH]], base=0, channel_multiplier=0,
                           allow_small_or_imprecise_dtypes=True)
            nc.gpsimd.iota(coarse, pattern=[[CH, LCN]], base=0, channel_multiplier=0,
                           allow_small_or_imprecise_dtypes=True)
            nc.gpsimd.iota(pos_f[:, :P1], pattern=[[1, P1]], base=0,
                           channel_multiplier=0, allow_small_or_imprecise_dtypes=True)

            def pos_piece_dve(lo, hi):
                nblk = (hi - lo) // CH
                finb = bass.AP(tensor=fine.tensor, offset=fine.offset,
                               ap=[fine.ap[0], [0, nblk], [1, CH]])
                corb = bass.AP(tensor=coarse.tensor,
                               offset=coarse.offset + (lo // CH) * coarse.ap[1][0],
                               ap=[coarse.ap[0], [coarse.ap[1][0], nblk], [0, CH]])
                nc.vector.tensor_tensor(
                    out=pos_f[:, lo:hi].rearrange("p (k f) -> p k f", f=CH),
                    in0=finb, in1=corb, op=OP.add)

            pos_piece_dve(P1, IW)
            pos_piece_dve(IW, IW2)
            # tail on GpSimd, concurrent with the DVE pieces
            nc.gpsimd.iota(pos_f[:, IW2:], pattern=[[1, L - IW2]], base=IW2,
                           channel_multiplier=0, allow_small_or_imprecise_dtypes=True)

            w_tiles = {}

            def wgen(b, t, cuts=None):
                bs, be = bands[b][t]
                wt = w_pools[t].tile([CH, wmax[t]], bf16)
                edges = [bs] + [c for c in (cuts or []) if bs < c < be] + [be]
                for lo, hi in zip(edges[:-1], edges[1:]):
                    # W = 2/sqrt(pi)*exp(-(s*pos - s*center)^2); the constant
                    # cancels via rd; scale+bias fold removes the mu stage.
                    nc.scalar.activation(
                        out=wt[:, lo - bs:hi - bs], in_=pos_f[:, lo:hi],
                        func=FT.Derivative_Erf,
                        scale=col(coefs_sb, cidx(0, b, t)),
                        bias=col(coefs_sb, cidx(1, b, t)),
                    )
                w_tiles[(b, t)] = wt

            # --- psum arena: one [CH, NSLOT, 512] allocation, manual
            # rotation chunk -> slot j%NSLOT; Tile range-deps do the rest.
            arena = psum_p.tile([CH, NSLOT, 512], f32)

            def group_ops(b, g, tail=False):
                sb = bands[b]
                j0 = (g % (NSLOT // GRP)) * GRP
                for k in range(GRP):
                    lo = (g * GRP + k) * CH
                    ctc = [t for t in range(TCN) if sb[t][0] <= lo and lo + CH <= sb[t][1]]
                    for i, t in enumerate(ctc):
                        off = lo - sb[t][0]
                        nc.tensor.matmul(
                            out=arena[:, j0 + k, :C + 1],
                            lhsT=w_tiles[(b, t)][:, off:off + CH],
                            rhs=xta_tiles[b][:, t, :],
                            start=(i == 0), stop=(i == len(ctc) - 1),
                        )
                dtmp = small_p.tile([CH, GRP], f32, tag="dtmp")
                nc.vector.tensor_scalar(
                    out=dtmp, in0=arena[:, j0:j0 + GRP, C],
                    scalar1=EPS_K, scalar2=None, op0=OP.add,
                )
                rd = small_p.tile([CH, GRP], f32, tag="rd")
                nc.vector.reciprocal(out=rd, in_=dtmp)
                osb = out_p.tile([CH, GRP, C], bf16, tag="osb")
                nd = 2 if tail else ND_PATTERN[b][g]
                rdb = bass.AP(tensor=rd.tensor, offset=rd.offset,
                              ap=[rd.ap[0], [rd.ap[1][0], nd], [0, C]])
                nc.vector.tensor_tensor(
                    out=osb[:, :nd, :], in0=arena[:, j0:j0 + nd, :C], in1=rdb,
                    op=OP.mult)
                for k in range(nd, GRP):
                    nc.scalar.activation(
                        out=osb[:, k, :], in_=arena[:, j0 + k, :C],
                        func=FT.Copy, scale=col(rd, k))
                if tail:
                    # drain: two half DMAs on independent queue rings
                    nc.sync.dma_start(out=out_d[b, g, :, :2, :], in_=osb[:, :2, :])
                    nc.scalar.dma_start(out=out_d[b, g, :, 2:, :], in_=osb[:, 2:, :])
                else:
                    nc.sync.dma_start(out=out_d[b, g], in_=osb)

            # --- schedule. Emission order per engine = execution order.
            # W tiles are emitted just-in-time: before the first group that
            # needs them (with one group of lookahead).
            stream = [(b, g) for b in range(BPC) for g in range(NT)]
            need = sorted(
                ((first_need(b, t), b, t) for b in range(BPC) for t in range(TCN)),
            )
            emitted = set()

            def emit_wgen_upto(pos_idx):
                for fn, b, t in need:
                    if fn <= pos_idx and (b, t) not in emitted:
                        emitted.add((b, t))
                        wgen(b, t, cuts=[P1, IW] if (b, t) == (0, 0) else None)

            emit_wgen_upto(1)          # everything needed by groups 0-1
            # PE warm-up on real data bridges the gap until the group stream
            # starts and keeps the clock ramping.
            for _ in range(NWARM):
                nc.tensor.matmul(
                    out=arena[:, NSLOT - 1, :C + 1],
                    lhsT=w_tiles[(0, 0)][:, :CH],
                    rhs=xta_tiles[0][:, 0, :], start=True, stop=True,
                )
            for t in range(TCN):
                load_xta_tc(1, t)
            for idx, (b, g) in enumerate(stream):
                group_ops(b, g, tail=(idx == len(stream) - 1))
                emit_wgen_upto(idx + 2)
    return nc


def _prepare_inputs(x, w, x_mask, y_mask, sigma_scale):
    center, s = _center_scale(w, sigma_scale[0])
    bands = _bands(center, w)
    nb = (-(s * center)).astype(np.float32)    # bias: -s*center (one f32 round)

    xm = np.broadcast_to(x_mask.reshape(B, T), (B, T)).astype(np.float32)
    if not np.all(xm == 1.0):
        x = (x * xm[:, None, :]).astype(np.float32)

    xt = np.ascontiguousarray(x.transpose(0, 2, 1))          # (B, T, C)
    xta = np.concatenate([xt, np.ones((B, T, 1), np.float32)], axis=2)
    # device layout [b, p, tc, c] for a descriptor-light DMA
    xta = np.ascontiguousarray(
        xta.reshape(B, TCN, CH, C + 1).transpose(0, 2, 1, 3)).astype(_bf16)

    in_maps = []
    for core in range(N_CORES):
        bsel = [core * BPC + s_ for s_ in range(BPC)]
        coefs = np.empty((2, BPC, TCN, CH), np.float32)
        for s_, bb in enumerate(bsel):
            coefs[0, s_] = s[bb].reshape(TCN, CH)
            coefs[1, s_] = nb[bb].reshape(TCN, CH)
        in_maps.append({
            "xta": xta[bsel],
            "coefs": np.ascontiguousarray(
                coefs.reshape(2 * BPC * TCN, CH).T),          # [CH, 16]
        })
    band_key = tuple(tuple(tuple(p) for p in sb) for sb in bands)
    return in_maps, band_key


def kernel(x, w, x_mask, y_mask, sigma_scale):
    x = np.asarray(x, dtype=np.float32)
    w = np.asarray(w, dtype=np.float32)
    x_mask = np.asarray(x_mask, dtype=np.float32)
    y_mask = np.asarray(y_mask, dtype=np.float32)
    sigma_scale = np.asarray(sigma_scale, dtype=np.float32)
    assert x.shape == (B, C, T) and w.shape == (B, T)

    in_maps, band_key = _prepare_inputs(x, w, x_mask, y_mask, sigma_scale)

    if band_key not in _cache:
        nc = _build(band_key)
        _split_excess_waits(nc)
        _cache[band_key] = nc
    nc = _cache[band_key]

    from concourse.bass_utils import run_bass_kernel_spmd

    res = run_bass_kernel_spmd(nc, in_maps, list(range(N_CORES)), trace=False)
    outs = []
    for i in range(N_CORES):
        o = np.asarray(res.results[i]["out"])                # (BPC, NT, CH, GRP, C)
        o = o.astype(np.float32).transpose(0, 1, 3, 2, 4).reshape(BPC, L, C)
        outs.append(o)
    full = np.concatenate(outs, axis=0).transpose(0, 2, 1)   # (B, C, L)
    ym = np.broadcast_to(y_mask.reshape(B, L), (B, L)).astype(np.float32)
    if not np.all(ym == 1.0):
        full = full * ym[:, None, :]
    return full


# revision 7
# speedup vs baseline: 1.4411x; 1.0144x over previous
"""DifferentiableLengthRegulator Trainium2 kernel.

out[b,c,l] = y_mask * (sum_t x[b,c,t]*W[b,t,l]) / (sum_t W[b,t,l] + eps)
W = exp(-0.5*(l - center[b,t])^2 / (w[b,t]^2*sigma_scale^2 + eps))

Sharding: data-parallel over batch B=16 -> 8 cores x 2 batches.
Per core, per batch (banded over the frame axis; Gaussian weights vanish
outside ~5 sigma of each token chunk's centers):
  ACT : W = DerivErf(s*pos + (-s*center)) -> bf16  (per-partition scale+bias
        fold computes mu inside the ACT op: no DVE mu stage at all)
  PE  : psum[l, 0:257] += W_tc[:, lslice]^T @ [xT | ones]
  DVE : d+eps = tensor_scalar(psum cols 256); rd = 1/(d+eps);
        evac psum*rd -> bf16 (ACT takes some chunks as Copy-scale)
PSUM is one manually-rotated [CH, 8, 512] arena (8 banks): chunk j lives in
slot j%8, giving depth-8 rotation with per-range dependencies instead of
depth-2 tile-pool rotation (PE never waits on evac).
Output layout [BPC, NT, CH, GRP, C] bf16 -> 2KB contiguous DMA lines; host
reshapes to (B, C, L) fp32. x_mask / y_mask folded on host.
"""

import numpy as np
import ml_dtypes

B, C, T, L = 16, 256, 512, 4096
N_CORES = 8
BPC = B // N_CORES  # batches per core
CH = 128            # partition chunk
TCN = T // CH       # 4 token chunks
LCN = L // CH       # 32 frame chunks
GRP = 4             # frame chunks per evac group
NT = LCN // GRP     # 8 groups per batch
NSLOT = 8           # psum arena slots (banks)
EPS = 1e-8
K_DERF = 1.1283791670955126  # 2/sqrt(pi), DerivErf's constant factor
MARGIN_SIGMA = 5.0
BAND_ALIGN = 128
NWARM = 4
LOOKAHEAD = 3

_bf16 = ml_dtypes.bfloat16
_cache = {}

# DVE evac chunks per group (out of GRP=4); remainder goes to ACT as
# Copy-scale. b0's groups run while ACT also generates later W tiles.
ND_PATTERN = {0: [3] * 8, 1: [3, 2, 3, 2, 3, 2, 3, 2]}


def _center_scale(w, sigma_scale):
    """Mirror the reference's cumsum/center math (same jax backend bits)."""
    try:
        import jax.numpy as jnp

        wj = jnp.asarray(w)
        center = np.asarray(jnp.cumsum(wj, axis=1) - 0.5 * wj, dtype=np.float32)
    except Exception:
        center = (np.cumsum(w, axis=1, dtype=np.float32) - 0.5 * w).astype(np.float32)
    sigma = (w * np.float32(sigma_scale)).astype(np.float32)
    # W = DerivErf(s*mu)*sqrt(pi)/2 = exp(-(s*mu)^2), s = sqrt(0.5/(sig^2+eps))
    s = np.sqrt(np.float32(0.5) / (np.square(sigma) + np.float32(EPS))).astype(np.float32)
    return center, s


def _bands(center, w_all):
    """Per (slot, tc) aligned frame band, unioned across cores (SPMD)."""
    bands = []
    for slot in range(BPC):
        rows = center[slot::BPC]      # the 8 batches that land in this slot
        wrows = w_all[slot::BPC]
        sb = []
        for tc in range(TCN):
            seg = rows[:, tc * CH:(tc + 1) * CH]
            margin = float(MARGIN_SIGMA * wrows[:, tc * CH:(tc + 1) * CH].max() + 1.0)
            bs = max(0, int(np.floor((seg.min() - margin) / BAND_ALIGN)) * BAND_ALIGN)
            be = min(L, int(np.ceil((seg.max() + margin) / BAND_ALIGN)) * BAND_ALIGN)
            if tc == 0:
                bs = 0
            if tc == TCN - 1:
                be = L
            bs = min(bs, be - CH)
            sb.append((bs, be))
        bands.append(sb)
    for sb in bands:
        for chunk in range(LCN):
            lo = chunk * CH
            assert any(bs <= lo and lo + CH <= be for bs, be in sb), (
                f"frame chunk {chunk} uncovered; widen MARGIN_SIGMA"
            )
    return bands


def _split_excess_waits(nc, max_waits=1):
    """walrus here caps sync-waits at 1 per compute instruction; move the
    excess onto injected same-engine NoOps just before the instruction
    (waiting earlier on the same engine is always safe)."""
    from concourse import mybir

    for f in nc.m.functions:
        for blk in f.blocks:
            new = []
            for inst in blk.instructions:
                si = inst.sync_info
                if si is not None and len(si.on_wait) > max_waits:
                    waits = list(si.on_wait)
                    keep, extra = waits[-max_waits:], waits[:-max_waits]
                    for i in range(0, len(extra), max_waits):
                        nop = mybir.InstNoOp(name=f"{inst.name}-xw{i}", ins=[], outs=[])
                        nop.engine = inst.engine
                        nop.sync_info = mybir.SyncInfo(
                            on_wait=extra[i:i + max_waits], on_update=[])
                        new.append(nop)
                    inst.sync_info = mybir.SyncInfo(
                        on_wait=keep, on_update=list(si.on_update))
                new.append(inst)
            blk.instructions = new


def _slim_tile_exit(tile):
    """Drop the second all-engine barrier in Tile's exit sequence: the
    sem-clears it orders are already completed by each engine finishing its
    own instruction stream before the NEFF ends (~4us saved)."""
    if getattr(tile.TileContext, "_slim_exit", False):
        return
    ScopedClock = tile.ScopedClock

    def _drain_and_barrier(self, tick_clock, wait_clock):
        drain_inst = self.nc.sync.drain()
        wait_clock.add_sem_waits(
            drain_inst.ins, ScopedClock({None: tick_clock.global_clock}))
        self.nc.all_engine_barrier()
        popped = self.nc._tile_sem_poison_stack.pop()
        assert popped is self._sem_poison
        self.nc.clear_and_free_semaphores(list(self.sems.allocated().values()))

    tile.TileContext._drain_and_barrier = _drain_and_barrier
    tile.TileContext._slim_exit = True


def _build(band_key):
    import concourse.bass as bass
    import concourse.tile as tile
    from concourse import mybir

    _slim_tile_exit(tile)
    bands = [[(band_key[s][t][0], band_key[s][t][1]) for t in range(TCN)]
             for s in range(BPC)]
    wmax = [max(bands[s][t][1] - bands[s][t][0] for s in range(BPC))
            for t in range(TCN)]

    nc = bass.Bass("TRN2", target_bir_lowering=False, debug=False)
    # xta host layout: [b, p, tc, c] so the DMA is descriptor-light
    xta_d = nc.declare_dram_parameter("xta", [BPC, CH, TCN, C + 1], mybir.dt.bfloat16, isOutput=False)
    coefs_d = nc.declare_dram_parameter("coefs", [CH, 2 * BPC * TCN], mybir.dt.float32, isOutput=False)
    out_d = nc.declare_dram_parameter("out", [BPC, NT, CH, GRP, C], mybir.dt.bfloat16, isOutput=True)

    f32 = mybir.dt.float32
    bf16 = mybir.dt.bfloat16
    FT = mybir.ActivationFunctionType
    OP = mybir.AluOpType
    EPS_K = float(EPS) * K_DERF

    # first group (in the b0..b1 stream) whose matmuls need W(b, t)
    def first_need(b, t):
        bs = bands[b][t][0]
        return b * NT + bs // (GRP * CH)

    with tile.TileContext(nc) as tc_:
        import contextlib

        with contextlib.ExitStack() as ctx:
            consts = ctx.enter_context(tc_.tile_pool(name="consts", bufs=1))
            xta_p = ctx.enter_context(tc_.tile_pool(name="xta", bufs=2))
            w_pools = [ctx.enter_context(tc_.tile_pool(name=f"w{t}", bufs=2)) for t in range(TCN)]
            psum_p = ctx.enter_context(tc_.tile_pool(name="ps", bufs=1, space="PSUM"))
            small_p = ctx.enter_context(tc_.tile_pool(name="small", bufs=6))
            out_p = ctx.enter_context(tc_.tile_pool(name="osb", bufs=4))

            def col(tile_, idx):
                return tile_[:, idx:idx + 1]

            def cidx(q, b, t):
                # q=0 -> s (scale), q=1 -> -s*center (bias)
                return (q * BPC + b) * TCN + t

            xta_tiles = {}

            def load_xta_tc(b, t):
                if b not in xta_tiles:
                    xta_sb = xta_p.tile([CH, TCN, C + 1], bf16, tag="xta")
                    xta_tiles[b] = xta_sb
                nc.sync.dma_start(out=xta_tiles[b][:, t, :], in_=xta_d[b, :, t, :])

            # --- startup: first xta piece, coefs, ACT table warm, pos build.
            load_xta_tc(0, 0)
            coefs_sb = consts.tile([CH, 2 * BPC * TCN], f32)
            nc.sync.dma_start(out=coefs_sb, in_=coefs_d[:, :])
            for t in range(1, TCN):
                load_xta_tc(0, t)

            warm = consts.tile([CH, 1], f32)
            nc.vector.memset(warm, 0.0)
            nc.scalar.activation(out=warm, in_=warm, func=FT.Derivative_Erf)

            # pos[l] = l, built cooperatively: GpSimd iota for the head and
            # tail pieces, DVE coarse+fine composed adds for the middle
            # (GpSimd iota is 1.8ns/col; DVE is 1.04 and idle before the
            # group stream starts).
            pos_f = consts.tile([CH, L], f32)
            fine = consts.tile([CH, CH], f32)
            coarse = consts.tile([CH, LCN], f32)
            P1 = 256
            IW = max(bands[s][0][1] for s in range(BPC))
            IW2 = max(bands[s][1][1] for s in range(BPC))
            nc.gpsimd.iota(fine, pattern=[[1, CH]], base=0, channel_multiplier=0,
                           allow_small_or_imprecise_dtypes=True)
            nc.gpsimd.iota(coarse, pattern=[[CH, LCN]], base=0, channel_multiplier=0,
                           allow_small_or_imprecise_dtypes=True)
            nc.gpsimd.iota(pos_f[:, :P1], pattern=[[1, P1]], base=0,
                           channel_multiplier=0, allow_small_or_imprecise_dtypes=True)

            def pos_piece_dve(lo, hi):
                nblk = (hi - lo) // CH
                finb = bass.AP(tensor=fine.tensor, offset=fine.offset,
                               ap=[fine.ap[0], [0, nblk], [1, CH]])
                corb = bass.AP(tensor=coarse.tensor,
                               offset=coarse.offset + (lo // CH) * coarse.ap[1][0],
                               ap=[coarse.ap[0], [coarse.ap[1][0], nblk], [0, CH]])
                nc.vector.tensor_tensor(
                    out=pos_f[:, lo:hi].rearrange("p (k f) -> p k f", f=CH),
                    in0=finb, in1=corb, op=OP.add)

            pos_piece_dve(P1, IW)
            pos_piece_dve(IW, IW2)
            # tail on GpSimd, concurrent with the DVE pieces
            nc.gpsimd.iota(pos_f[:, IW2:], pattern=[[1, L - IW2]], base=IW2,
                           channel_multiplier=0, allow_small_or_imprecise_dtypes=True)

            w_tiles = {}

            def wgen(b, t, cuts=None):
                bs, be = bands[b][t]
                wt = w_pools[t].tile([CH, wmax[t]], bf16)
                edges = [bs] + [c for c in (cuts or []) if bs < c < be] + [be]
                for lo, hi in zip(edges[:-1], edges[1:]):
                    # W = 2/sqrt(pi)*exp(-(s*pos - s*center)^2); the constant
                    # cancels via rd; scale+bias fold removes the mu stage.
                    nc.scalar.activation(
                        out=wt[:, lo - bs:hi - bs], in_=pos_f[:, lo:hi],
                        func=FT.Derivative_Erf,
                        scale=col(coefs_sb, cidx(0, b, t)),
                        bias=col(coefs_sb, cidx(1, b, t)),
                    )
                w_tiles[(b, t)] = wt

            # --- psum arena: one [CH, NSLOT, 512] allocation, manual
            # rotation chunk -> slot j%NSLOT; Tile range-deps do the rest.
            arena = psum_p.tile([CH, NSLOT, 512], f32)

            def chunk_matmuls(b, g, k):
                sb = bands[b]
                j0 = (g % (NSLOT // GRP)) * GRP
                lo = (g * GRP + k) * CH
                ctc = [t for t in range(TCN) if sb[t][0] <= lo and lo + CH <= sb[t][1]]
                for i, t in enumerate(ctc):
                    off = lo - sb[t][0]
                    nc.tensor.matmul(
                        out=arena[:, j0 + k, :C + 1],
                        lhsT=w_tiles[(b, t)][:, off:off + CH],
                        rhs=xta_tiles[b][:, t, :],
                        start=(i == 0), stop=(i == len(ctc) - 1),
                    )

            def group_ops(b, g, nd):
                j0 = (g % (NSLOT // GRP)) * GRP
                for k in range(GRP):
                    chunk_matmuls(b, g, k)
                dtmp = small_p.tile([CH, GRP], f32, tag="dtmp")
                nc.vector.tensor_scalar(
                    out=dtmp, in0=arena[:, j0:j0 + GRP, C],
                    scalar1=EPS_K, scalar2=None, op0=OP.add,
                )
                rd = small_p.tile([CH, GRP], f32, tag="rd")
                nc.vector.reciprocal(out=rd, in_=dtmp)
                osb = out_p.tile([CH, GRP, C], bf16, tag="osb")
                rdb = bass.AP(tensor=rd.tensor, offset=rd.offset,
                              ap=[rd.ap[0], [rd.ap[1][0], nd], [0, C]])
                nc.vector.tensor_tensor(
                    out=osb[:, :nd, :], in0=arena[:, j0:j0 + nd, :C], in1=rdb,
                    op=OP.mult)
                for k in range(nd, GRP):
                    nc.scalar.activation(
                        out=osb[:, k, :], in_=arena[:, j0 + k, :C],
                        func=FT.Copy, scale=col(rd, k))
                nc.sync.dma_start(out=out_d[b, g], in_=osb)

            def tail_group(b, g):
                # drain: process in 2-chunk halves so half 0's evac+DMA
                # overlap half 1's matmuls; halves on independent queue rings
                j0 = (g % (NSLOT // GRP)) * GRP
                osb = out_p.tile([CH, GRP, C], bf16, tag="osb")
                for h in range(2):
                    for k in (2 * h, 2 * h + 1):
                        chunk_matmuls(b, g, k)
                    dtmp = small_p.tile([CH, 2], f32, tag="dtmp")
                    nc.vector.tensor_scalar(
                        out=dtmp, in0=arena[:, j0 + 2 * h:j0 + 2 * h + 2, C],
                        scalar1=EPS_K, scalar2=None, op0=OP.add,
                    )
                    rd = small_p.tile([CH, 2], f32, tag="rd")
                    nc.vector.reciprocal(out=rd, in_=dtmp)
                    rdb = bass.AP(tensor=rd.tensor, offset=rd.offset,
                                  ap=[rd.ap[0], [rd.ap[1][0], 1], [0, C]])
                    nc.vector.tensor_tensor(
                        out=osb[:, 2 * h, :], in0=arena[:, j0 + 2 * h, :C],
                        in1=rdb, op=OP.mult)
                    nc.scalar.activation(
                        out=osb[:, 2 * h + 1, :], in_=arena[:, j0 + 2 * h + 1, :C],
                        func=FT.Copy, scale=col(rd, 1))
                    eng = nc.sync if h == 0 else nc.scalar
                    eng.dma_start(out=out_d[b, g, :, 2 * h:2 * h + 2, :],
                                  in_=osb[:, 2 * h:2 * h + 2, :])

            # --- schedule. Emission order per engine = execution order.
            # W tiles are emitted just-in-time, LOOKAHEAD groups before first
            # use; groups whose ACT copies would queue behind a DERF run
            # their whole evac on the DVE (nd=GRP).
            stream = [(b, g) for b in range(BPC) for g in range(NT)]
            need = sorted(
                ((first_need(b, t), b, t) for b in range(BPC) for t in range(TCN)),
            )
            emitted = set()

            def wgens_due(pos_idx):
                due = []
                for fn, b, t in need:
                    if fn <= pos_idx and (b, t) not in emitted:
                        emitted.add((b, t))
                        due.append((b, t))
                return due

            plan = []        # ('w', b, t) | ('g', b, g)
            for (b, t) in wgens_due(1):
                plan.append(('w', b, t))
            for idx, (b, g) in enumerate(stream):
                plan.append(('g', b, g))
                for (b2, t2) in wgens_due(idx + LOOKAHEAD):
                    plan.append(('w', b2, t2))

            for kind, x, y in plan:
                if kind == 'w':
                    wgen(x, y, cuts=[P1, IW] if (x, y) == (0, 0) else None)
                    if (x, y) == (0, 0):
                        # PE warm-up on real data bridges the pre-stream gap
                        for _ in range(NWARM):
                            nc.tensor.matmul(
                                out=arena[:, NSLOT - 1, :C + 1],
                                lhsT=w_tiles[(0, 0)][:, :CH],
                                rhs=xta_tiles[0][:, 0, :], start=True, stop=True,
                            )
                        for t in range(TCN):
                            load_xta_tc(1, t)
                else:
                    idx = plan.index((kind, x, y))
                    if (x, y) == stream[-1]:
                        tail_group(x, y)
                    else:
                        nxt = plan[idx + 1:idx + 2]
                        nd = GRP if (nxt and nxt[0][0] == 'w') else ND_PATTERN[x][y]
                        group_ops(x, y, nd)
    return nc


def _prepare_inputs(x, w, x_mask, y_mask, sigma_scale):
    center, s = _center_scale(w, sigma_scale[0])
    bands = _bands(center, w)
    nb = (-(s * center)).astype(np.float32)    # bias: -s*center (one f32 round)

    xm = np.broadcast_to(x_mask.reshape(B, T), (B, T)).astype(np.float32)
    if not np.all(xm == 1.0):
        x = (x * xm[:, None, :]).astype(np.float32)

    xt = np.ascontiguousarray(x.transpose(0, 2, 1))          # (B, T, C)
    xta = np.concatenate([xt, np.ones((B, T, 1), np.float32)], axis=2)
    # device layout [b, p, tc, c] for a descriptor-light DMA
    xta = np.ascontiguousarray(
        xta.reshape(B, TCN, CH, C + 1).transpose(0, 2, 1, 3)).astype(_bf16)

    in_maps = []
    for core in range(N_CORES):
        bsel = [core * BPC + s_ for s_ in range(BPC)]
        coefs = np.empty((2, BPC, TCN, CH), np.float32)
        for s_, bb in enumerate(bsel):
            coefs[0, s_] = s[bb].reshape(TCN, CH)
            coefs[1, s_] = nb[bb].reshape(TCN, CH)
        in_maps.append({
            "xta": xta[bsel],
            "coefs": np.ascontiguousarray(
                coefs.reshape(2 * BPC * TCN, CH).T),          # [CH, 16]
        })
    band_key = tuple(tuple(tuple(p) for p in sb) for sb in bands)
    return in_maps, band_key


def kernel(x, w, x_mask, y_mask, sigma_scale):
    x = np.asarray(x, dtype=np.float32)
    w = np.asarray(w, dtype=np.float32)
    x_mask = np.asarray(x_mask, dtype=np.float32)
    y_mask = np.asarray(y_mask, dtype=np.float32)
    sigma_scale = np.asarray(sigma_scale, dtype=np.float32)
    assert x.shape == (B, C, T) and w.shape == (B, T)

    in_maps, band_key = _prepare_inputs(x, w, x_mask, y_mask, sigma_scale)

    if band_key not in _cache:
        nc = _build(band_key)
        _split_excess_waits(nc)
        _cache[band_key] = nc
    nc = _cache[band_key]

    from concourse.bass_utils import run_bass_kernel_spmd

    res = run_bass_kernel_spmd(nc, in_maps, list(range(N_CORES)), trace=False)
    outs = []
    for i in range(N_CORES):
        o = np.asarray(res.results[i]["out"])                # (BPC, NT, CH, GRP, C)
        o = o.astype(np.float32).transpose(0, 1, 3, 2, 4).reshape(BPC, L, C)
        outs.append(o)
    full = np.concatenate(outs, axis=0).transpose(0, 2, 1)   # (B, C, L)
    ym = np.broadcast_to(y_mask.reshape(B, L), (B, L)).astype(np.float32)
    if not np.all(ym == 1.0):
        full = full * ym[:, None, :]
    return full
